# revision 4
# baseline (speedup 1.0000x reference)
"""Distributed GCN (2x GCNConv + Linear) on 8 Trainium2 NeuronCores via Bass/Tile.

Algorithm (matches the PyG-style reference):
  h1 = relu(gcnconv(x, W1, b1, mask1));  h2 = relu(gcnconv(h1, W2, b2, mask2))
  out = h2 @ Wl + bl
where gcnconv(x, W, b, keep) with self-loops:
  h = x @ W;  deg = segsum(keep, dst) + 1;  dis = rsqrt(deg)
  out = segsum(h[src] * (keep * dis[src] * dis[dst]), dst) + h * dis^2 + b

Distribution: nodes padded to N_PAD = 8 * SHARD, contiguous node shard per
core.  Edges partitioned by dst core.  Per layer: each core computes H for
its shard (TensorE, kept resident in SBUF), AllGather makes full H available
in every core's DRAM (bf16, two sections so the int16 gather indices stay in
range and the collective pipelines), then per 128-edge block the core
bulk-gathers H[src] rows with dma_gather (SWDGE, round-robin over the 4
queues; all of a layer's gathers are issued section-major up front so the
four queues stay saturated), builds the scaled one-hot segment matrix
M[e, d] = coef_e * (dstloc_e == d) ON DEVICE with a single fused DVE
tensor_scalar (is_equal then mult against an iota row), and accumulates
out^T[f, d] += G_blk^T @ M_blk on TensorE in PSUM.  The self-loop block
multiplies the SBUF-resident own-shard H tile against diag(dis^2), built on
DVE from an identity tile, so it needs no DMA at all.  ReLU+bias runs on
ScalarE straight out of PSUM four tiles at a time, the next layer's
H-matmul follows per tile, and H/out stores go to DRAM in four-tile batches.

Host-side numpy does graph preprocessing only (edge partitioning, padding,
degree/normalization scalars, index layout); all O(N*F) / O(E*F) float
work runs on the NeuronCores.
"""

import numpy as np
import ml_dtypes

import concourse.bass as bass
import concourse.bacc as bacc
import concourse.tile as tile
import concourse.mybir as mybir
from concourse.bass_utils import run_bass_kernel_spmd

P = 128
N_CORES = 8

# Full-problem dimensions (hardcoded per the task contract).
N_NODES = 50000
F_IN = 128
F_HID = 128
F_OUT = 64

# Gather chunking: one dma_gather covers <= SUB_B 128-edge blocks.
SUB_B = 24
# SWDGE queues to rotate gathers over (4 Q7 core pairs).
N_QUEUES = 4
# src sections per shard (pipelined AllGather + int16 idx range).
N_SEC = 2
# dst tiles per PSUM batch.
QUAD = 4


# ---------------------------------------------------------------------------
# Host-side preprocessing
# ---------------------------------------------------------------------------

class _LayerLayout:
    __slots__ = ("chunks", "tile_segs", "n_blocks", "blk2chunk")

    def __init__(self):
        self.chunks = []        # (sec, b0, nblk)
        self.tile_segs = []     # per tile: [(sec, b0, nb)]
        self.n_blocks = 0
        self.blk2chunk = None   # block -> chunk index


def _prep_layer(src_k, dst_k, coef_k, n_pad, shard, sub_b):
    """Shared static layout + per-core device arrays for one layer.

    src_k/dst_k/coef_k: kept (mask=1) edges (self-loops handled separately).
    Returns (_LayerLayout, per_core list of dicts with idx/dl/cf).
    """
    tiles_pc = shard // P
    sec_rows = shard // N_SEC

    core = dst_k // shard
    ttl = (dst_k % shard) // P
    dloc = (dst_k % P).astype(np.float32)
    sec = (src_k % shard) // sec_rows
    sidx = (src_k // shard) * sec_rows + (src_k % shard) % sec_rows
    cf = coef_k.astype(np.float32)

    key = (core * N_SEC + sec) * tiles_pc + ttl
    order = np.argsort(key, kind="stable")
    sidx, dloc, cf, key = sidx[order], dloc[order], cf[order], key[order]
    bnd = np.searchsorted(key, np.arange(N_CORES * N_SEC * tiles_pc + 1))
    cnt = (bnd[1:] - bnd[:-1]).reshape(N_CORES, N_SEC, tiles_pc)
    bcnt = -(-cnt.max(axis=0) // P)            # [N_SEC, tiles_pc]

    lay = _LayerLayout()
    lay.tile_segs = [[] for _ in range(tiles_pc)]
    bucket_b0 = np.zeros((N_SEC, tiles_pc), dtype=np.int64)
    blocks_sec = []
    nb_tot = 0
    for s in range(N_SEC):
        for tt in range(tiles_pc):
            nb = int(bcnt[s, tt])
            if nb == 0:
                continue
            bucket_b0[s, tt] = nb_tot
            lay.tile_segs[tt].append((s, nb_tot, nb))
            blocks_sec.extend([s] * nb)
            nb_tot += nb
    lay.n_blocks = nb_tot

    lay.blk2chunk = np.zeros(max(nb_tot, 1), dtype=np.int64)
    b = 0
    while b < nb_tot:
        s = blocks_sec[b]
        e = b
        while e < nb_tot and blocks_sec[e] == s and e - b < sub_b:
            e += 1
        lay.blk2chunk[b:e] = len(lay.chunks)
        lay.chunks.append((s, b, e - b))
        b = e

    per_core = []
    for c in range(N_CORES):
        idxf = np.zeros(max(nb_tot, 1) * P, dtype=np.int16)
        dla = np.zeros((P, max(nb_tot, 1)), dtype=np.float32)
        cfa = np.zeros((P, max(nb_tot, 1)), dtype=np.float32)
        for s in range(N_SEC):
            for tt in range(tiles_pc):
                i = (c * N_SEC + s) * tiles_pc + tt
                a, bb = bnd[i], bnd[i + 1]
                n_e = bb - a
                if n_e == 0:
                    continue
                pos = bucket_b0[s, tt] * P + np.arange(n_e)
                idxf[pos] = sidx[a:bb].astype(np.int16)
                dla[pos % P, pos // P] = dloc[a:bb]
                cfa[pos % P, pos // P] = cf[a:bb]
        w = idxf.reshape(-1, 16).T                     # [16, nb*8]
        per_core.append({
            "idx": np.ascontiguousarray(np.tile(w, (8, 1))),
            "dl": dla,
            "cf": cfa,
        })
    return lay, per_core


def _prepare(x, edge_index, mask1, mask2, W1, b1, W2, b2, Wl, bl,
             n, n_pad, sub_b=SUB_B):
    """Full host prep: returns (static_layouts, in_maps)."""
    shard = n_pad // N_CORES
    tiles_pc = shard // P
    assert shard % P == 0
    src = np.asarray(edge_index[0], dtype=np.int64)
    dst = np.asarray(edge_index[1], dtype=np.int64)

    bf16 = ml_dtypes.bfloat16

    layouts = []
    layer_data = []
    selfws = []
    for mask in (np.asarray(mask1), np.asarray(mask2)):
        keep = mask.astype(bool)
        ks, kd = src[keep], dst[keep]
        deg = np.bincount(kd, minlength=n).astype(np.float64) + 1.0
        dis = 1.0 / np.sqrt(deg)
        coef_k = (dis[ks] * dis[kd]).astype(np.float32)
        selfw = np.zeros((n_pad,), dtype=np.float32)
        selfw[:n] = (dis * dis).astype(np.float32)
        lay, pc = _prep_layer(ks, kd, coef_k, n_pad, shard, sub_b)
        layouts.append(lay)
        layer_data.append(pc)
        selfws.append(selfw)

    xp = np.zeros((n_pad, F_IN), dtype=np.float32)
    xp[:n] = np.asarray(x, dtype=np.float32)

    in_maps = []
    for c in range(N_CORES):
        m = {
            "xt": np.ascontiguousarray(
                xp[c * shard:(c + 1) * shard].T).astype(bf16),
            "w1": np.asarray(W1, np.float32).astype(bf16),
            "w2": np.asarray(W2, np.float32).astype(bf16),
            "wl": np.asarray(Wl, np.float32).astype(bf16),
            "b1c": np.asarray(b1, np.float32).reshape(P, 1),
            "b2c": np.asarray(b2, np.float32).reshape(P, 1),
            "blbc": np.broadcast_to(np.asarray(bl, np.float32),
                                    (P, F_OUT)).copy(),
            "ident": np.eye(P, dtype=np.float32).astype(bf16),
        }
        for li in (0, 1):
            d = layer_data[li][c]
            m[f"idx{li+1}"] = d["idx"]
            m[f"dl{li+1}"] = d["dl"]
            m[f"cf{li+1}"] = d["cf"]
            sw = selfws[li][c * shard:(c + 1) * shard]
            m[f"sw{li+1}"] = np.ascontiguousarray(
                sw.reshape(tiles_pc, P).T.astype(np.float32))
        in_maps.append(m)
    return layouts, in_maps


# ---------------------------------------------------------------------------
# Device program
# ---------------------------------------------------------------------------

def _build(layouts, n_pad):
    shard = n_pad // N_CORES
    tiles_pc = shard // P
    sec_rows = shard // N_SEC
    gdt = mybir.dt.bfloat16
    f32 = mybir.dt.float32

    nc = bacc.Bacc("TRN2", target_bir_lowering=False, debug=False,
                   num_swdge_queues=N_QUEUES)

    xt_d = nc.declare_dram_parameter("xt", [P, shard], gdt, isOutput=False)
    w1_d = nc.declare_dram_parameter("w1", [P, F_HID], gdt, isOutput=False)
    w2_d = nc.declare_dram_parameter("w2", [P, F_HID], gdt, isOutput=False)
    wl_d = nc.declare_dram_parameter("wl", [P, F_OUT], gdt, isOutput=False)
    b1c_d = nc.declare_dram_parameter("b1c", [P, 1], f32, isOutput=False)
    b2c_d = nc.declare_dram_parameter("b2c", [P, 1], f32, isOutput=False)
    blbc_d = nc.declare_dram_parameter("blbc", [P, F_OUT], f32, isOutput=False)
    ident_d = nc.declare_dram_parameter("ident", [P, P], gdt, isOutput=False)
    idx_d, dl_d, cf_d, sw_d = [], [], [], []
    for li, lay in enumerate(layouts):
        nb = max(lay.n_blocks, 1)
        idx_d.append(nc.declare_dram_parameter(
            f"idx{li+1}", [P, nb * 8], mybir.dt.int16, isOutput=False))
        dl_d.append(nc.declare_dram_parameter(
            f"dl{li+1}", [P, nb], f32, isOutput=False))
        cf_d.append(nc.declare_dram_parameter(
            f"cf{li+1}", [P, nb], f32, isOutput=False))
        sw_d.append(nc.declare_dram_parameter(
            f"sw{li+1}", [P, tiles_pc], f32, isOutput=False))
    out_d = nc.declare_dram_parameter("out", [shard, F_OUT], f32, isOutput=True)

    h_shard = [nc.dram_tensor(f"h{li}_shard", [shard, P], gdt)
               for li in (1, 2)]
    h_sec = [[nc.dram_tensor(f"h{li}_sec{s}", [N_CORES * sec_rows, P], gdt,
                             addr_space="Shared") for s in range(N_SEC)]
             for li in (1, 2)]

    rg = [list(range(N_CORES))]
    relu = mybir.ActivationFunctionType.Relu
    copyf = mybir.ActivationFunctionType.Copy
    is_eq = mybir.AluOpType.is_equal
    mult = mybir.AluOpType.mult
    qctr = [0]

    n_quads = -(-tiles_pc // QUAD)

    with tile.TileContext(nc) as tc:
        with (
            tc.tile_pool(name="consts", bufs=1) as cpool,
            tc.tile_pool(name="gbuf", bufs=14) as gpool,
            tc.tile_pool(name="mpool", bufs=6) as mpool,
            tc.tile_pool(name="opool", bufs=3) as opool,
            tc.tile_pool(name="aggp", bufs=4, space="PSUM") as aggpool,
            tc.tile_pool(name="hp", bufs=3, space="PSUM") as hpool,
        ):
            def load_const(dram, shape, dt):
                t = cpool.tile(shape, dt, tag=dram.name)
                nc.sync.dma_start(t[:], dram[:])
                return t

            xt_sb = load_const(xt_d, [P, shard], gdt)
            w1_sb = load_const(w1_d, [P, F_HID], gdt)
            w2_sb = load_const(w2_d, [P, F_HID], gdt)
            wl_sb = load_const(wl_d, [P, F_OUT], gdt)
            b1c_sb = load_const(b1c_d, [P, 1], f32)
            b2c_sb = load_const(b2c_d, [P, 1], f32)
            blbc_sb = load_const(blbc_d, [P, F_OUT], f32)
            ident_sb = load_const(ident_d, [P, P], gdt)
            idx_sb = [load_const(idx_d[li], [P, max(layouts[li].n_blocks, 1) * 8],
                                 mybir.dt.int16) for li in (0, 1)]
            dl_sb = [load_const(dl_d[li], [P, max(layouts[li].n_blocks, 1)], f32)
                     for li in (0, 1)]
            cf_sb = [load_const(cf_d[li], [P, max(layouts[li].n_blocks, 1)], f32)
                     for li in (0, 1)]
            sw_sb = [load_const(sw_d[li], [P, tiles_pc], f32) for li in (0, 1)]
            iota_sb = cpool.tile([P, P], gdt, tag="iota")
            nc.gpsimd.iota(iota_sb[:], pattern=[[1, P]], base=0,
                           channel_multiplier=0,
                           allow_small_or_imprecise_dtypes=True)
            # persistent own-shard H (bf16), tile-major columns
            hall1 = cpool.tile([P, shard], gdt, tag="hall1")
            hall2 = cpool.tile([P, shard], gdt, tag="hall2")
            hall = [hall1, hall2]

            def quad_tiles(q):
                return list(range(q * QUAD, min((q + 1) * QUAD, tiles_pc)))

            def store_h(li, q, src_ap):
                """Batched SBUF->DRAM store of a quad of H tiles + AllGather."""
                tts = quad_tiles(q)
                rows = slice(tts[0] * P, (tts[-1] + 1) * P)
                dst = h_shard[li][rows, :].rearrange(
                    "(k p) f -> p k f", p=P)
                nc.sync.dma_start(dst, src_ap)
                for s in range(N_SEC):
                    hi = (s + 1) * sec_rows
                    if tts[0] * P < hi <= (tts[-1] + 1) * P:
                        nc.gpsimd.collective_compute(
                            "AllGather", mybir.AluOpType.bypass,
                            replica_groups=rg,
                            ins=[h_shard[li][s * sec_rows:(s + 1) * sec_rows, :]],
                            outs=[h_sec[li][s][:]])

            # ---- phase 0: H1 = X @ W1 (per-shard), sectioned AllGather ----
            for q in range(n_quads):
                tts = quad_tiles(q)
                nq = len(tts)
                hp = hpool.tile([P, QUAD * F_HID], f32, tag="hp")
                for k, tt in enumerate(tts):
                    nc.tensor.matmul(out=hp[:, k * F_HID:(k + 1) * F_HID],
                                     lhsT=xt_sb[:, tt * P:(tt + 1) * P],
                                     rhs=w1_sb[:], start=True, stop=True)
                hsl = hall[0][:, tts[0] * P:(tts[-1] + 1) * P]
                nc.scalar.activation(out=hsl, in_=hp[:, :nq * F_HID],
                                     func=copyf)
                store_h(0, q, hsl.rearrange("p (k f) -> p k f", f=P))

            # ---- aggregation layers ----
            for li in (0, 1):
                lay = layouts[li]
                bcol = b1c_sb if li == 0 else b2c_sb
                w_next = w2_sb if li == 0 else wl_sb
                n_next = F_HID if li == 0 else F_OUT

                # issue all gathers section-major; queues rotate
                gbufs = []
                for (s, b0, nblk) in lay.chunks:
                    gb = gpool.tile([P, SUB_B, P], gdt, tag="gb")
                    ni = nblk * P
                    nc.gpsimd.dma_gather(
                        gb[:, :nblk, :], h_sec[li][s][:],
                        idx_sb[li][:, b0 * 8:b0 * 8 + nblk * 8],
                        ni, ni, P, single_packet=False,
                        queue_num=qctr[0] % N_QUEUES)
                    qctr[0] += 1
                    gbufs.append(gb)

                for q in range(n_quads):
                    tts = quad_tiles(q)
                    nq = len(tts)
                    aggp = aggpool.tile([P, QUAD * P], f32, tag="aggp")
                    for k, tt in enumerate(tts):
                        sl = slice(k * P, (k + 1) * P)
                        first = True
                        for (s, b0, nb) in lay.tile_segs[tt]:
                            for b in range(b0, b0 + nb):
                                ci = int(lay.blk2chunk[b])
                                jj = b - lay.chunks[ci][1]
                                m = mpool.tile([P, P], gdt, tag="m")
                                nc.vector.tensor_scalar(
                                    m[:], iota_sb[:],
                                    dl_sb[li][:, b:b + 1],
                                    cf_sb[li][:, b:b + 1],
                                    is_eq, mult)
                                nc.tensor.matmul(
                                    out=aggp[:, sl],
                                    lhsT=gbufs[ci][:, jj, :], rhs=m[:],
                                    start=first, stop=False)
                                first = False
                        # self-loop: own-shard H tile x diag(dis^2)
                        d = mpool.tile([P, P], gdt, tag="d")
                        nc.vector.tensor_scalar(
                            d[:], ident_sb[:], sw_sb[li][:, tt:tt + 1],
                            None, mult)
                        nc.tensor.matmul(
                            out=aggp[:, sl],
                            lhsT=hall[li][:, tt * P:(tt + 1) * P], rhs=d[:],
                            start=first, stop=True)
                    # relu(agg + b) in transposed layout (bias per-part)
                    outT = opool.tile([P, QUAD * P], gdt, tag="outT")
                    nc.scalar.activation(out=outT[:, :nq * P],
                                         in_=aggp[:, :nq * P],
                                         func=relu, bias=bcol[:])
                    hp2 = hpool.tile([P, QUAD * F_HID], f32, tag="hp")
                    for k, tt in enumerate(tts):
                        nc.tensor.matmul(
                            out=hp2[:, k * n_next:(k + 1) * n_next],
                            lhsT=outT[:, k * P:(k + 1) * P],
                            rhs=w_next[:], start=True, stop=True)
                    if li == 0:
                        hsl = hall[1][:, tts[0] * P:(tts[-1] + 1) * P]
                        nc.scalar.activation(out=hsl, in_=hp2[:, :nq * F_HID],
                                             func=copyf)
                        store_h(1, q, hsl.rearrange("p (k f) -> p k f", f=P))
                    else:
                        osb = opool.tile([P, QUAD * F_OUT], f32, tag="osb")
                        for k in range(nq):
                            nc.vector.tensor_tensor(
                                out=osb[:, k * F_OUT:(k + 1) * F_OUT],
                                in0=hp2[:, k * F_OUT:(k + 1) * F_OUT],
                                in1=blbc_sb[:], op=mybir.AluOpType.add)
                        rows = slice(tts[0] * P, (tts[-1] + 1) * P)
                        dst = out_d[rows, :].rearrange("(k p) f -> p k f", p=P)
                        nc.sync.dma_start(
                            dst, osb[:, :nq * F_OUT].rearrange(
                                "p (k f) -> p k f", f=F_OUT))

    nc.compile()
    return nc


# ---------------------------------------------------------------------------
# Entry point
# ---------------------------------------------------------------------------

def _run(x, edge_index, mask1, mask2, W1, b1, W2, b2, Wl, bl,
         n, n_pad, lo_limit=None):
    layouts, in_maps = _prepare(x, edge_index, mask1, mask2,
                                W1, b1, W2, b2, Wl, bl, n, n_pad)
    nc = _build(layouts, n_pad)
    res = run_bass_kernel_spmd(nc, in_maps, core_ids=list(range(N_CORES)))
    out = np.concatenate([res.results[c]["out"] for c in range(N_CORES)],
                         axis=0)
    return out[:n].astype(np.float32)


def kernel(x, edge_index, mask1, mask2, W1, b1, W2, b2, Wl, bl):
    n_pad = 50176  # 8 cores * 49 tiles * 128
    return _run(x, edge_index, mask1, mask2, W1, b1, W2, b2, Wl, bl,
                N_NODES, n_pad)


# revision 8
# speedup vs baseline: 1.0622x; 1.0622x over previous
"""Distributed GCN (2x GCNConv + Linear) on 8 Trainium2 NeuronCores via Bass/Tile.

Algorithm (matches the PyG-style reference):
  h1 = relu(gcnconv(x, W1, b1, mask1));  h2 = relu(gcnconv(h1, W2, b2, mask2))
  out = h2 @ Wl + bl
where gcnconv(x, W, b, keep) with self-loops:
  h = x @ W;  deg = segsum(keep, dst) + 1;  dis = rsqrt(deg)
  out = segsum(h[src] * (keep * dis[src] * dis[dst]), dst) + h * dis^2 + b

The edge coefficient factorizes: coef_e = dis[src_e] * dis[dst_e].  The
kernel exploits this: the per-shard H tiles are scaled by dis on the way out
of PSUM (free per-partition scale slot of the ScalarE copy), so the gathered
rows are already Hhat = dis*H; the segment matrices become PURE 0/1 one-hots
(padding rows use dstloc=-1 so they match nothing); and the remaining
dis[dst] factor is one per-quad DVE multiply on the PSUM accumulator before
the ReLU.  The self-loop term dis^2*H = dis[dst]*Hhat[dst] falls out of the
same structure as a plain identity matmul against the SBUF-resident Hhat.

Distribution: nodes padded to N_PAD = 8 * SHARD, contiguous node shard per
core.  Edges partitioned by dst core.  Per layer: each core computes Hhat
for its shard (TensorE, kept resident in SBUF), AllGather makes full Hhat
available in every core's DRAM (bf16, two sections so the int16 gather
indices stay in range and the collective pipelines), then per 128-edge block
the core bulk-gathers Hhat[src] rows with dma_gather (SWDGE, round-robin
over the 4 queues; all of a layer's gathers are issued section-major up
front so the four queues stay saturated), builds the one-hot segment
matrices in PANELS of MB blocks with a single broadcast DVE is_equal
against an iota table, and accumulates out^T[f, d] += G_blk^T @ M_blk on
TensorE in PSUM.  ReLU+bias runs on ScalarE straight out of PSUM four tiles
at a time, the next layer's H-matmul follows per tile, and H/out stores go
to DRAM in four-tile batches.

Host-side numpy does graph preprocessing only (edge partitioning, padding,
degree/normalization scalars, index layout); all O(N*F) / O(E*F) float
work runs on the NeuronCores.
"""

import numpy as np
import ml_dtypes

import concourse.bass as bass
import concourse.bacc as bacc
import concourse.tile as tile
import concourse.mybir as mybir
from concourse.bass_utils import run_bass_kernel_spmd

P = 128
N_CORES = 8

# Full-problem dimensions (hardcoded per the task contract).
N_NODES = 50000
F_IN = 128
F_HID = 128
F_OUT = 64

# Gather chunking: one dma_gather covers <= SUB_B 128-edge blocks.
SUB_B = 24
# SWDGE queues to rotate gathers over (4 Q7 core pairs).
N_QUEUES = 4
# src sections per shard (pipelined AllGather + int16 idx range).
N_SEC = 2
# dst tiles per PSUM batch.
QUAD = 4
# one-hot segment-matrix panel width (blocks per DVE is_equal).
MB = 32


# ---------------------------------------------------------------------------
# Host-side preprocessing
# ---------------------------------------------------------------------------

class _LayerLayout:
    __slots__ = ("chunks", "tile_segs", "n_blocks", "blk2chunk", "panels",
                 "blk2panel")

    def __init__(self):
        self.chunks = []        # (sec, b0, nblk)
        self.tile_segs = []     # per tile: [(sec, b0, nb)]
        self.n_blocks = 0
        self.blk2chunk = None   # block -> chunk index
        self.panels = []        # (b0, nb)
        self.blk2panel = None


def _prep_layer(src_k, dst_k, n_pad, shard, sub_b):
    """Shared static layout + per-core device arrays for one layer.

    src_k/dst_k: kept (mask=1) edges (self-loops handled separately).
    Returns (_LayerLayout, per_core list of dicts with idx/dl).
    """
    tiles_pc = shard // P
    sec_rows = shard // N_SEC

    core = dst_k // shard
    ttl = (dst_k % shard) // P
    dloc = (dst_k % P).astype(np.float32)
    sec = (src_k % shard) // sec_rows
    sidx = (src_k // shard) * sec_rows + (src_k % shard) % sec_rows

    key = (core * N_SEC + sec) * tiles_pc + ttl
    order = np.argsort(key, kind="stable")
    sidx, dloc, key = sidx[order], dloc[order], key[order]
    bnd = np.searchsorted(key, np.arange(N_CORES * N_SEC * tiles_pc + 1))
    cnt = (bnd[1:] - bnd[:-1]).reshape(N_CORES, N_SEC, tiles_pc)
    bcnt = -(-cnt.max(axis=0) // P)            # [N_SEC, tiles_pc]

    lay = _LayerLayout()
    lay.tile_segs = [[] for _ in range(tiles_pc)]
    bucket_b0 = np.zeros((N_SEC, tiles_pc), dtype=np.int64)
    blocks_sec = []
    nb_tot = 0
    for s in range(N_SEC):
        for tt in range(tiles_pc):
            nb = int(bcnt[s, tt])
            if nb == 0:
                continue
            bucket_b0[s, tt] = nb_tot
            lay.tile_segs[tt].append((s, nb_tot, nb))
            blocks_sec.extend([s] * nb)
            nb_tot += nb
    lay.n_blocks = nb_tot

    lay.blk2chunk = np.zeros(max(nb_tot, 1), dtype=np.int64)
    b = 0
    while b < nb_tot:
        s = blocks_sec[b]
        e = b
        while e < nb_tot and blocks_sec[e] == s and e - b < sub_b:
            e += 1
        lay.blk2chunk[b:e] = len(lay.chunks)
        lay.chunks.append((s, b, e - b))
        b = e

    # one-hot panels: restart at section boundaries so both per-section
    # streams advance monotonically with tile order
    lay.blk2panel = np.zeros(max(nb_tot, 1), dtype=np.int64)
    b = 0
    while b < nb_tot:
        s = blocks_sec[b]
        e = b
        while e < nb_tot and blocks_sec[e] == s and e - b < MB:
            e += 1
        lay.blk2panel[b:e] = len(lay.panels)
        lay.panels.append((b, e - b))
        b = e

    per_core = []
    for c in range(N_CORES):
        idxf = np.zeros(max(nb_tot, 1) * P, dtype=np.int16)
        dla = np.full((P, max(nb_tot, 1)), -1.0, dtype=np.float32)
        for s in range(N_SEC):
            for tt in range(tiles_pc):
                i = (c * N_SEC + s) * tiles_pc + tt
                a, bb = bnd[i], bnd[i + 1]
                n_e = bb - a
                if n_e == 0:
                    continue
                pos = bucket_b0[s, tt] * P + np.arange(n_e)
                idxf[pos] = sidx[a:bb].astype(np.int16)
                dla[pos % P, pos // P] = dloc[a:bb]
        w = idxf.reshape(-1, 16).T                     # [16, nb*8]
        per_core.append({
            "idx": np.ascontiguousarray(np.tile(w, (8, 1))),
            "dl": dla.astype(ml_dtypes.bfloat16),
        })
    return lay, per_core


def _prepare(x, edge_index, mask1, mask2, W1, b1, W2, b2, Wl, bl,
             n, n_pad, sub_b=SUB_B):
    """Full host prep: returns (static_layouts, in_maps)."""
    shard = n_pad // N_CORES
    tiles_pc = shard // P
    assert shard % P == 0
    src = np.asarray(edge_index[0], dtype=np.int64)
    dst = np.asarray(edge_index[1], dtype=np.int64)

    bf16 = ml_dtypes.bfloat16

    layouts = []
    layer_data = []
    diss = []
    for mask in (np.asarray(mask1), np.asarray(mask2)):
        keep = mask.astype(bool)
        ks, kd = src[keep], dst[keep]
        deg = np.bincount(kd, minlength=n).astype(np.float64) + 1.0
        dis = np.zeros((n_pad,), dtype=np.float32)
        dis[:n] = (1.0 / np.sqrt(deg)).astype(np.float32)
        lay, pc = _prep_layer(ks, kd, n_pad, shard, sub_b)
        layouts.append(lay)
        layer_data.append(pc)
        diss.append(dis)

    xp = np.zeros((n_pad, F_IN), dtype=np.float32)
    xp[:n] = np.asarray(x, dtype=np.float32)

    in_maps = []
    for c in range(N_CORES):
        m = {
            "xt": np.ascontiguousarray(
                xp[c * shard:(c + 1) * shard].T).astype(bf16),
            "w1": np.asarray(W1, np.float32).astype(bf16),
            "w2": np.asarray(W2, np.float32).astype(bf16),
            "wl": np.asarray(Wl, np.float32).astype(bf16),
            "b1c": np.asarray(b1, np.float32).reshape(P, 1),
            "b2c": np.asarray(b2, np.float32).reshape(P, 1),
            "blbc": np.broadcast_to(np.asarray(bl, np.float32),
                                    (P, F_OUT)).copy(),
            "ident": np.eye(P, dtype=np.float32).astype(bf16),
        }
        for li in (0, 1):
            d = layer_data[li][c]
            m[f"idx{li+1}"] = d["idx"]
            m[f"dl{li+1}"] = d["dl"]
            dsh = diss[li][c * shard:(c + 1) * shard]
            # per-tile columns of dis (ScalarE Hhat-copy scales)
            m[f"sw{li+1}"] = np.ascontiguousarray(
                dsh.reshape(tiles_pc, P).T.astype(np.float32))
            # dis along the free dim (per-quad dst scale), rows identical
            m[f"dq{li+1}"] = np.broadcast_to(dsh, (P, shard)).copy()
        in_maps.append(m)
    return layouts, in_maps


# ---------------------------------------------------------------------------
# Device program
# ---------------------------------------------------------------------------

def _build(layouts, n_pad):
    shard = n_pad // N_CORES
    tiles_pc = shard // P
    sec_rows = shard // N_SEC
    gdt = mybir.dt.bfloat16
    f32 = mybir.dt.float32

    nc = bacc.Bacc("TRN2", target_bir_lowering=False, debug=False,
                   num_swdge_queues=N_QUEUES)

    xt_d = nc.declare_dram_parameter("xt", [P, shard], gdt, isOutput=False)
    w1_d = nc.declare_dram_parameter("w1", [P, F_HID], gdt, isOutput=False)
    w2_d = nc.declare_dram_parameter("w2", [P, F_HID], gdt, isOutput=False)
    wl_d = nc.declare_dram_parameter("wl", [P, F_OUT], gdt, isOutput=False)
    b1c_d = nc.declare_dram_parameter("b1c", [P, 1], f32, isOutput=False)
    b2c_d = nc.declare_dram_parameter("b2c", [P, 1], f32, isOutput=False)
    blbc_d = nc.declare_dram_parameter("blbc", [P, F_OUT], f32, isOutput=False)
    ident_d = nc.declare_dram_parameter("ident", [P, P], gdt, isOutput=False)
    idx_d, dl_d, sw_d, dq_d = [], [], [], []
    for li, lay in enumerate(layouts):
        nb = max(lay.n_blocks, 1)
        idx_d.append(nc.declare_dram_parameter(
            f"idx{li+1}", [P, nb * 8], mybir.dt.int16, isOutput=False))
        dl_d.append(nc.declare_dram_parameter(
            f"dl{li+1}", [P, nb], gdt, isOutput=False))
        sw_d.append(nc.declare_dram_parameter(
            f"sw{li+1}", [P, tiles_pc], f32, isOutput=False))
        dq_d.append(nc.declare_dram_parameter(
            f"dq{li+1}", [P, shard], f32, isOutput=False))
    out_d = nc.declare_dram_parameter("out", [shard, F_OUT], f32, isOutput=True)

    h_shard = [nc.dram_tensor(f"h{li}_shard", [shard, P], gdt)
               for li in (1, 2)]
    h_sec = [[nc.dram_tensor(f"h{li}_sec{s}", [N_CORES * sec_rows, P], gdt,
                             addr_space="Shared") for s in range(N_SEC)]
             for li in (1, 2)]

    rg = [list(range(N_CORES))]
    relu = mybir.ActivationFunctionType.Relu
    copyf = mybir.ActivationFunctionType.Copy
    is_eq = mybir.AluOpType.is_equal
    mult = mybir.AluOpType.mult
    qctr = [0]

    n_quads = -(-tiles_pc // QUAD)

    with tile.TileContext(nc) as tc:
        with (
            tc.tile_pool(name="consts", bufs=1) as cpool,
            tc.tile_pool(name="gbuf", bufs=11) as gpool,
            tc.tile_pool(name="ppool", bufs=4) as ppool,
            tc.tile_pool(name="opool", bufs=3) as opool,
            tc.tile_pool(name="aggp", bufs=4, space="PSUM") as aggpool,
            tc.tile_pool(name="hp", bufs=3, space="PSUM") as hpool,
        ):
            def load_const(dram, shape, dt):
                t = cpool.tile(shape, dt, tag=dram.name)
                nc.sync.dma_start(t[:], dram[:])
                return t

            xt_sb = load_const(xt_d, [P, shard], gdt)
            w1_sb = load_const(w1_d, [P, F_HID], gdt)
            w2_sb = load_const(w2_d, [P, F_HID], gdt)
            wl_sb = load_const(wl_d, [P, F_OUT], gdt)
            b1c_sb = load_const(b1c_d, [P, 1], f32)
            b2c_sb = load_const(b2c_d, [P, 1], f32)
            blbc_sb = load_const(blbc_d, [P, F_OUT], f32)
            ident_sb = load_const(ident_d, [P, P], gdt)
            idx_sb = [load_const(idx_d[li], [P, max(layouts[li].n_blocks, 1) * 8],
                                 mybir.dt.int16) for li in (0, 1)]
            dl_sb = [load_const(dl_d[li], [P, max(layouts[li].n_blocks, 1)], gdt)
                     for li in (0, 1)]
            sw_sb = [load_const(sw_d[li], [P, tiles_pc], f32) for li in (0, 1)]
            iota_sb = cpool.tile([P, MB * P], gdt, tag="iota")
            nc.gpsimd.iota(iota_sb[:].rearrange("p (b f) -> p b f", f=P),
                           pattern=[[0, MB], [1, P]], base=0,
                           channel_multiplier=0,
                           allow_small_or_imprecise_dtypes=True)
            # persistent own-shard Hhat (bf16), tile-major columns
            hall1 = cpool.tile([P, shard], gdt, tag="hall1")
            hall2 = cpool.tile([P, shard], gdt, tag="hall2")
            hall = [hall1, hall2]

            def quad_tiles(q):
                return list(range(q * QUAD, min((q + 1) * QUAD, tiles_pc)))

            def store_h(li, q, src_ap):
                """Batched SBUF->DRAM store of a quad of H tiles + AllGather."""
                tts = quad_tiles(q)
                rows = slice(tts[0] * P, (tts[-1] + 1) * P)
                dst = h_shard[li][rows, :].rearrange(
                    "(k p) f -> p k f", p=P)
                nc.sync.dma_start(dst, src_ap)
                for s in range(N_SEC):
                    hi = (s + 1) * sec_rows
                    if tts[0] * P < hi <= (tts[-1] + 1) * P:
                        nc.gpsimd.collective_compute(
                            "AllGather", mybir.AluOpType.bypass,
                            replica_groups=rg,
                            ins=[h_shard[li][s * sec_rows:(s + 1) * sec_rows, :]],
                            outs=[h_sec[li][s][:]])

            # ---- phase 0: Hhat1 = dis1*(X @ W1) per shard, AllGather ----
            for q in range(n_quads):
                tts = quad_tiles(q)
                nq = len(tts)
                hp = hpool.tile([P, QUAD * F_HID], f32, tag="hp")
                for k, tt in enumerate(tts):
                    nc.tensor.matmul(out=hp[:, k * F_HID:(k + 1) * F_HID],
                                     lhsT=xt_sb[:, tt * P:(tt + 1) * P],
                                     rhs=w1_sb[:], start=True, stop=True)
                    nc.scalar.activation(
                        out=hall[0][:, tt * P:(tt + 1) * P],
                        in_=hp[:, k * F_HID:(k + 1) * F_HID],
                        func=copyf, scale=sw_sb[0][:, tt:tt + 1])
                hsl = hall[0][:, tts[0] * P:(tts[-1] + 1) * P]
                store_h(0, q, hsl.rearrange("p (k f) -> p k f", f=P))

            # ---- aggregation layers ----
            # f32 accumulator for the section-0 sweep partials
            acc = cpool.tile([P, shard], f32, tag="acc")
            for li in (0, 1):
                lay = layouts[li]
                bcol = b1c_sb if li == 0 else b2c_sb
                w_next = w2_sb if li == 0 else wl_sb
                n_next = F_HID if li == 0 else F_OUT

                gbufs = {}

                def gather(ci):
                    if ci not in gbufs:
                        (s, b0, nblk) = lay.chunks[ci]
                        gb = gpool.tile([P, SUB_B, P], gdt, tag="gb")
                        ni = nblk * P
                        nc.gpsimd.dma_gather(
                            gb[:, :nblk, :], h_sec[li][s][:],
                            idx_sb[li][:, b0 * 8:b0 * 8 + nblk * 8],
                            ni, ni, P, single_packet=False,
                            queue_num=qctr[0] % N_QUEUES)
                        qctr[0] += 1
                        gbufs[ci] = gb
                    return gbufs[ci]

                # one-hot panels, built on first use
                pbufs = {}

                def panel(pi):
                    if pi not in pbufs:
                        b0, nb = lay.panels[pi]
                        mp = ppool.tile([P, MB * P], gdt, tag="m")
                        nc.vector.tensor_tensor(
                            out=mp[:, :nb * P].rearrange(
                                "p (b f) -> p b f", f=P),
                            in0=iota_sb[:, :nb * P].rearrange(
                                "p (b f) -> p b f", f=P),
                            in1=dl_sb[li][:, b0:b0 + nb].broadcast_to(
                                (P, nb, P)),
                            op=is_eq)
                        pbufs[pi] = mp
                    return pbufs[pi]

                def blocks_of(tt, sweep_s):
                    for (s, b0, nb) in lay.tile_segs[tt]:
                        if s == sweep_s:
                            for b in range(b0, b0 + nb):
                                yield b

                def emit_blocks(tt, sweep_s, aggp, sl, close):
                    """Emit matmuls for tile tt's sweep_s blocks; if close,
                    the last one gets stop=True.  Returns True if none."""
                    blks = list(blocks_of(tt, sweep_s))
                    for i, b in enumerate(blks):
                        ci = int(lay.blk2chunk[b])
                        jj = b - lay.chunks[ci][1]
                        pi = int(lay.blk2panel[b])
                        mp = panel(pi)
                        gb = gather(ci)
                        pj = b - lay.panels[pi][0]
                        nc.tensor.matmul(
                            out=aggp[:, sl], lhsT=gb[:, jj, :],
                            rhs=mp[:, pj * P:(pj + 1) * P],
                            start=(i == 0),
                            stop=(close and i == len(blks) - 1))
                    return len(blks) == 0

                # sweep 0: section-0 contributions -> f32 accumulator
                for q in range(n_quads):
                    tts = quad_tiles(q)
                    nq = len(tts)
                    aggp = aggpool.tile([P, QUAD * P], f32, tag="aggp")
                    for k, tt in enumerate(tts):
                        sl = slice(k * P, (k + 1) * P)
                        if emit_blocks(tt, 0, aggp, sl, close=True):
                            nc.vector.memset(aggp[:, sl], 0.0)
                    cols = slice(tts[0] * P, (tts[-1] + 1) * P)
                    nc.scalar.activation(out=acc[:, cols],
                                         in_=aggp[:, :nq * P], func=copyf)
                # sweep 1: section-1 + self-loop + carry-in, finalize
                for q in range(n_quads):
                    tts = quad_tiles(q)
                    nq = len(tts)
                    aggp = aggpool.tile([P, QUAD * P], f32, tag="aggp")
                    for k, tt in enumerate(tts):
                        sl = slice(k * P, (k + 1) * P)
                        first = emit_blocks(tt, 1, aggp, sl, close=False)
                        nc.tensor.matmul(
                            out=aggp[:, sl],
                            lhsT=hall[li][:, tt * P:(tt + 1) * P],
                            rhs=ident_sb[:], start=first, stop=True)
                    cols = slice(tts[0] * P, (tts[-1] + 1) * P)
                    # add the section-0 carry-in, then dis[dst] scale
                    nc.vector.tensor_tensor(
                        out=aggp[:, :nq * P], in0=aggp[:, :nq * P],
                        in1=acc[:, cols], op=mybir.AluOpType.add)
                    dqq = opool.tile([P, QUAD * P], f32, tag="dqq")
                    nc.sync.dma_start(dqq[:, :nq * P], dq_d[li][:, cols])
                    nc.vector.tensor_tensor(
                        out=aggp[:, :nq * P], in0=aggp[:, :nq * P],
                        in1=dqq[:, :nq * P], op=mult)
                    outT = opool.tile([P, QUAD * P], gdt, tag="outT")
                    nc.scalar.activation(out=outT[:, :nq * P],
                                         in_=aggp[:, :nq * P],
                                         func=relu, bias=bcol[:])
                    hp2 = hpool.tile([P, QUAD * F_HID], f32, tag="hp")
                    for k, tt in enumerate(tts):
                        nc.tensor.matmul(
                            out=hp2[:, k * n_next:(k + 1) * n_next],
                            lhsT=outT[:, k * P:(k + 1) * P],
                            rhs=w_next[:], start=True, stop=True)
                    if li == 0:
                        for k, tt in enumerate(tts):
                            nc.scalar.activation(
                                out=hall[1][:, tt * P:(tt + 1) * P],
                                in_=hp2[:, k * F_HID:(k + 1) * F_HID],
                                func=copyf, scale=sw_sb[1][:, tt:tt + 1])
                        hsl = hall[1][:, tts[0] * P:(tts[-1] + 1) * P]
                        store_h(1, q, hsl.rearrange("p (k f) -> p k f", f=P))
                    else:
                        osb = opool.tile([P, QUAD * F_OUT], f32, tag="osb")
                        for k in range(nq):
                            nc.vector.tensor_tensor(
                                out=osb[:, k * F_OUT:(k + 1) * F_OUT],
                                in0=hp2[:, k * F_OUT:(k + 1) * F_OUT],
                                in1=blbc_sb[:], op=mybir.AluOpType.add)
                        rows = slice(tts[0] * P, (tts[-1] + 1) * P)
                        dst = out_d[rows, :].rearrange("(k p) f -> p k f", p=P)
                        nc.sync.dma_start(
                            dst, osb[:, :nq * F_OUT].rearrange(
                                "p (k f) -> p k f", f=F_OUT))

    nc.compile()
    return nc


# ---------------------------------------------------------------------------
# Entry point
# ---------------------------------------------------------------------------

def _run(x, edge_index, mask1, mask2, W1, b1, W2, b2, Wl, bl,
         n, n_pad, lo_limit=None):
    layouts, in_maps = _prepare(x, edge_index, mask1, mask2,
                                W1, b1, W2, b2, Wl, bl, n, n_pad)
    nc = _build(layouts, n_pad)
    res = run_bass_kernel_spmd(nc, in_maps, core_ids=list(range(N_CORES)))
    out = np.concatenate([res.results[c]["out"] for c in range(N_CORES)],
                         axis=0)
    return out[:n].astype(np.float32)


def kernel(x, edge_index, mask1, mask2, W1, b1, W2, b2, Wl, bl):
    n_pad = 50176  # 8 cores * 49 tiles * 128
    return _run(x, edge_index, mask1, mask2, W1, b1, W2, b2, Wl, bl,
                N_NODES, n_pad)


# revision 13
# speedup vs baseline: 1.1019x; 1.0373x over previous
"""Distributed GCN (2x GCNConv + Linear) on 8 Trainium2 NeuronCores via Bass/Tile.

Algorithm (matches the PyG-style reference):
  h1 = relu(gcnconv(x, W1, b1, mask1));  h2 = relu(gcnconv(h1, W2, b2, mask2))
  out = h2 @ Wl + bl
where gcnconv(x, W, b, keep) with self-loops:
  h = x @ W;  deg = segsum(keep, dst) + 1;  dis = rsqrt(deg)
  out = segsum(h[src] * (keep * dis[src] * dis[dst]), dst) + h * dis^2 + b

The edge coefficient factorizes: coef_e = dis[src_e] * dis[dst_e].  The
kernel exploits this: the per-shard H tiles are scaled by dis on the way out
of PSUM (free per-partition scale slot of the ScalarE copy), so the gathered
rows are already Hhat = dis*H; the segment matrices become PURE 0/1 one-hots
(padding rows use dstloc=-1 so they match nothing); and the remaining
dis[dst] factor is one per-quad DVE multiply on the PSUM accumulator before
the ReLU.  The self-loop term dis^2*H = dis[dst]*Hhat[dst] falls out of the
same structure as a plain identity matmul against the SBUF-resident Hhat.

Distribution: nodes padded to N_PAD = 8 * SHARD, contiguous node shard per
core.  Edges partitioned by dst core.  Per layer: each core computes Hhat
for its shard (TensorE, kept resident in SBUF), AllGather makes full Hhat
available in every core's DRAM (bf16, two sections so the int16 gather
indices stay in range and the collective pipelines), then per 128-edge block
the core bulk-gathers Hhat[src] rows with dma_gather (SWDGE, round-robin
over the 4 queues; all of a layer's gathers are issued section-major up
front so the four queues stay saturated), builds the one-hot segment
matrices in PANELS of MB blocks with a single broadcast DVE is_equal
against an iota table, and accumulates out^T[f, d] += G_blk^T @ M_blk on
TensorE in PSUM.  ReLU+bias runs on ScalarE straight out of PSUM four tiles
at a time, the next layer's H-matmul follows per tile, and H/out stores go
to DRAM in four-tile batches.

Host-side numpy does graph preprocessing only (edge partitioning, padding,
degree/normalization scalars, index layout); all O(N*F) / O(E*F) float
work runs on the NeuronCores.
"""

import numpy as np
import ml_dtypes

import concourse.bass as bass
import concourse.bacc as bacc
import concourse.tile as tile
import concourse.mybir as mybir
from concourse.bass_utils import run_bass_kernel_spmd

P = 128
N_CORES = 8

# Full-problem dimensions (hardcoded per the task contract).
N_NODES = 50000
F_IN = 128
F_HID = 128
F_OUT = 64

# Gather chunking: one dma_gather covers <= SUB_B 128-edge blocks.
SUB_B = 24
# SWDGE queues to rotate gathers over (4 Q7 core pairs).
N_QUEUES = 4
# src sections per shard (pipelined AllGather + int16 idx range).
N_SEC = 2
# dst tiles per PSUM batch.
QUAD = 4
# one-hot segment-matrix panel width (blocks per DVE is_equal).
MB = 32


# ---------------------------------------------------------------------------
# Host-side preprocessing
# ---------------------------------------------------------------------------

class _LayerLayout:
    __slots__ = ("chunks", "tile_segs", "n_blocks", "blk2chunk", "panels",
                 "blk2panel")

    def __init__(self):
        self.chunks = []        # (sec, b0, nblk)
        self.tile_segs = []     # per tile: [(sec, b0, nb)]
        self.n_blocks = 0
        self.blk2chunk = None   # block -> chunk index
        self.panels = []        # (b0, nb)
        self.blk2panel = None


def _prep_layer(src_k, dst_k, n_pad, shard, sub_b):
    """Shared static layout + per-core device arrays for one layer.

    src_k/dst_k: kept (mask=1) edges (self-loops handled separately).
    Returns (_LayerLayout, per_core list of dicts with idx/dl).
    """
    tiles_pc = shard // P
    sec_rows = shard // N_SEC

    core = dst_k // shard
    ttl = (dst_k % shard) // P
    dloc = (dst_k % P).astype(np.float32)
    sec = (src_k % shard) // sec_rows
    sidx = (src_k // shard) * sec_rows + (src_k % shard) % sec_rows

    key = (core * N_SEC + sec) * tiles_pc + ttl
    order = np.argsort(key, kind="stable")
    sidx, dloc, key = sidx[order], dloc[order], key[order]
    bnd = np.searchsorted(key, np.arange(N_CORES * N_SEC * tiles_pc + 1))
    cnt = (bnd[1:] - bnd[:-1]).reshape(N_CORES, N_SEC, tiles_pc)
    bcnt = -(-cnt.max(axis=0) // P)            # [N_SEC, tiles_pc]

    lay = _LayerLayout()
    lay.tile_segs = [[] for _ in range(tiles_pc)]
    bucket_b0 = np.zeros((N_SEC, tiles_pc), dtype=np.int64)
    blocks_sec = []
    nb_tot = 0
    for s in range(N_SEC):
        for tt in range(tiles_pc):
            nb = int(bcnt[s, tt])
            if nb == 0:
                continue
            bucket_b0[s, tt] = nb_tot
            lay.tile_segs[tt].append((s, nb_tot, nb))
            blocks_sec.extend([s] * nb)
            nb_tot += nb
    lay.n_blocks = nb_tot

    lay.blk2chunk = np.zeros(max(nb_tot, 1), dtype=np.int64)
    b = 0
    while b < nb_tot:
        s = blocks_sec[b]
        e = b
        while e < nb_tot and blocks_sec[e] == s and e - b < sub_b:
            e += 1
        lay.blk2chunk[b:e] = len(lay.chunks)
        lay.chunks.append((s, b, e - b))
        b = e

    # one-hot panels: restart at section boundaries so both per-section
    # streams advance monotonically with tile order
    lay.blk2panel = np.zeros(max(nb_tot, 1), dtype=np.int64)
    b = 0
    while b < nb_tot:
        s = blocks_sec[b]
        e = b
        while e < nb_tot and blocks_sec[e] == s and e - b < MB:
            e += 1
        lay.blk2panel[b:e] = len(lay.panels)
        lay.panels.append((b, e - b))
        b = e

    per_core = []
    for c in range(N_CORES):
        idxf = np.zeros(max(nb_tot, 1) * P, dtype=np.int16)
        dla = np.full((P, max(nb_tot, 1)), -1.0, dtype=np.float32)
        for s in range(N_SEC):
            for tt in range(tiles_pc):
                i = (c * N_SEC + s) * tiles_pc + tt
                a, bb = bnd[i], bnd[i + 1]
                n_e = bb - a
                if n_e == 0:
                    continue
                pos = bucket_b0[s, tt] * P + np.arange(n_e)
                idxf[pos] = sidx[a:bb].astype(np.int16)
                dla[pos % P, pos // P] = dloc[a:bb]
        w = idxf.reshape(-1, 16).T                     # [16, nb*8]
        per_core.append({
            "idx": np.ascontiguousarray(np.tile(w, (8, 1))),
            "dl": dla.astype(ml_dtypes.bfloat16),
        })
    return lay, per_core


def _prepare(x, edge_index, mask1, mask2, W1, b1, W2, b2, Wl, bl,
             n, n_pad, sub_b=SUB_B):
    """Full host prep: returns (static_layouts, in_maps)."""
    shard = n_pad // N_CORES
    tiles_pc = shard // P
    assert shard % P == 0
    src = np.asarray(edge_index[0], dtype=np.int64)
    dst = np.asarray(edge_index[1], dtype=np.int64)

    bf16 = ml_dtypes.bfloat16

    layouts = []
    layer_data = []
    diss = []
    for mask in (np.asarray(mask1), np.asarray(mask2)):
        keep = mask.astype(bool)
        ks, kd = src[keep], dst[keep]
        deg = np.bincount(kd, minlength=n).astype(np.float64) + 1.0
        dis = np.zeros((n_pad,), dtype=np.float32)
        dis[:n] = (1.0 / np.sqrt(deg)).astype(np.float32)
        lay, pc = _prep_layer(ks, kd, n_pad, shard, sub_b)
        layouts.append(lay)
        layer_data.append(pc)
        diss.append(dis)

    xp = np.zeros((n_pad, F_IN), dtype=np.float32)
    xp[:n] = np.asarray(x, dtype=np.float32)

    in_maps = []
    for c in range(N_CORES):
        m = {
            "xt": np.ascontiguousarray(
                xp[c * shard:(c + 1) * shard].T).astype(bf16),
            "w1": np.asarray(W1, np.float32).astype(bf16),
            "w2": np.asarray(W2, np.float32).astype(bf16),
            "wl": np.asarray(Wl, np.float32).astype(bf16),
            "b1c": np.asarray(b1, np.float32).reshape(P, 1),
            "b2c": np.asarray(b2, np.float32).reshape(P, 1),
            "blbc": np.broadcast_to(np.asarray(bl, np.float32),
                                    (P, F_OUT)).copy(),
            "ident": np.eye(P, dtype=np.float32).astype(bf16),
        }
        for li in (0, 1):
            d = layer_data[li][c]
            m[f"idx{li+1}"] = d["idx"]
            m[f"dl{li+1}"] = d["dl"]
            dsh = diss[li][c * shard:(c + 1) * shard]
            # per-tile columns of dis (ScalarE Hhat-copy scales)
            m[f"sw{li+1}"] = np.ascontiguousarray(
                dsh.reshape(tiles_pc, P).T.astype(np.float32))
            # dis along the free dim (per-quad dst scale), rows identical
            m[f"dq{li+1}"] = np.broadcast_to(dsh, (P, shard)).copy()
        in_maps.append(m)
    return layouts, in_maps


# ---------------------------------------------------------------------------
# Device program
# ---------------------------------------------------------------------------

def _build(layouts, n_pad):
    shard = n_pad // N_CORES
    tiles_pc = shard // P
    sec_rows = shard // N_SEC
    gdt = mybir.dt.bfloat16
    f32 = mybir.dt.float32

    nc = bacc.Bacc("TRN2", target_bir_lowering=False, debug=False,
                   num_swdge_queues=N_QUEUES)

    xt_d = nc.declare_dram_parameter("xt", [P, shard], gdt, isOutput=False)
    w1_d = nc.declare_dram_parameter("w1", [P, F_HID], gdt, isOutput=False)
    w2_d = nc.declare_dram_parameter("w2", [P, F_HID], gdt, isOutput=False)
    wl_d = nc.declare_dram_parameter("wl", [P, F_OUT], gdt, isOutput=False)
    b1c_d = nc.declare_dram_parameter("b1c", [P, 1], f32, isOutput=False)
    b2c_d = nc.declare_dram_parameter("b2c", [P, 1], f32, isOutput=False)
    blbc_d = nc.declare_dram_parameter("blbc", [P, F_OUT], f32, isOutput=False)
    ident_d = nc.declare_dram_parameter("ident", [P, P], gdt, isOutput=False)
    idx_d, dl_d, sw_d, dq_d = [], [], [], []
    for li, lay in enumerate(layouts):
        nb = max(lay.n_blocks, 1)
        idx_d.append(nc.declare_dram_parameter(
            f"idx{li+1}", [P, nb * 8], mybir.dt.int16, isOutput=False))
        dl_d.append(nc.declare_dram_parameter(
            f"dl{li+1}", [P, nb], gdt, isOutput=False))
        sw_d.append(nc.declare_dram_parameter(
            f"sw{li+1}", [P, tiles_pc], f32, isOutput=False))
        dq_d.append(nc.declare_dram_parameter(
            f"dq{li+1}", [P, shard], f32, isOutput=False))
    out_d = nc.declare_dram_parameter("out", [shard, F_OUT], f32, isOutput=True)

    h_shard = [nc.dram_tensor(f"h{li}_shard", [shard, P], gdt)
               for li in (1, 2)]
    h_sec = [[nc.dram_tensor(f"h{li}_sec{s}", [N_CORES * sec_rows, P], gdt,
                             addr_space="Shared") for s in range(N_SEC)]
             for li in (1, 2)]

    rg = [list(range(N_CORES))]
    relu = mybir.ActivationFunctionType.Relu
    copyf = mybir.ActivationFunctionType.Copy
    is_eq = mybir.AluOpType.is_equal
    mult = mybir.AluOpType.mult
    qctr = [0]

    n_quads = -(-tiles_pc // QUAD)

    with tile.TileContext(nc) as tc:
        with (
            tc.tile_pool(name="consts", bufs=1) as cpool,
            tc.tile_pool(name="gbuf", bufs=11) as gpool,
            tc.tile_pool(name="ppool", bufs=4) as ppool,
            tc.tile_pool(name="opool", bufs=3) as opool,
            tc.tile_pool(name="aggp", bufs=4, space="PSUM") as aggpool,
            tc.tile_pool(name="hp", bufs=3, space="PSUM") as hpool,
        ):
            def load_const(dram, shape, dt):
                t = cpool.tile(shape, dt, tag=dram.name)
                nc.sync.dma_start(t[:], dram[:])
                return t

            xt_sb = load_const(xt_d, [P, shard], gdt)
            w1_sb = load_const(w1_d, [P, F_HID], gdt)
            w2_sb = load_const(w2_d, [P, F_HID], gdt)
            wl_sb = load_const(wl_d, [P, F_OUT], gdt)
            b1c_sb = load_const(b1c_d, [P, 1], f32)
            b2c_sb = load_const(b2c_d, [P, 1], f32)
            blbc_sb = load_const(blbc_d, [P, F_OUT], f32)
            ident_sb = load_const(ident_d, [P, P], gdt)
            idx_sb = [load_const(idx_d[li], [P, max(layouts[li].n_blocks, 1) * 8],
                                 mybir.dt.int16) for li in (0, 1)]
            dl_sb = [load_const(dl_d[li], [P, max(layouts[li].n_blocks, 1)], gdt)
                     for li in (0, 1)]
            sw_sb = [load_const(sw_d[li], [P, tiles_pc], f32) for li in (0, 1)]
            iota_sb = cpool.tile([P, MB * P], gdt, tag="iota")
            nc.gpsimd.iota(iota_sb[:].rearrange("p (b f) -> p b f", f=P),
                           pattern=[[0, MB], [1, P]], base=0,
                           channel_multiplier=0,
                           allow_small_or_imprecise_dtypes=True)
            # persistent own-shard Hhat (bf16), tile-major columns
            hall1 = cpool.tile([P, shard], gdt, tag="hall1")
            hall2 = cpool.tile([P, shard], gdt, tag="hall2")
            hall = [hall1, hall2]

            def quad_tiles(q):
                return list(range(q * QUAD, min((q + 1) * QUAD, tiles_pc)))

            # tiles overlapping each section (sections are row ranges)
            sec_tiles = []
            for s in range(N_SEC):
                lo, hi = s * sec_rows, (s + 1) * sec_rows
                sec_tiles.append({tt for tt in range(tiles_pc)
                                  if tt * P < hi and (tt + 1) * P > lo})
            remaining = [None, None]

            def store_h(li, q, src_ap):
                """Batched SBUF->DRAM store of a quad of H tiles + AllGather."""
                tts = quad_tiles(q)
                rows = slice(tts[0] * P, (tts[-1] + 1) * P)
                dst = h_shard[li][rows, :].rearrange(
                    "(k p) f -> p k f", p=P)
                nc.sync.dma_start(dst, src_ap)
                for s in range(N_SEC):
                    if remaining[li][s] is None:
                        continue
                    remaining[li][s] -= set(tts)
                    if not remaining[li][s]:
                        remaining[li][s] = None
                        nc.gpsimd.collective_compute(
                            "AllGather", mybir.AluOpType.bypass,
                            replica_groups=rg,
                            ins=[h_shard[li][s * sec_rows:(s + 1) * sec_rows, :]],
                            outs=[h_sec[li][s][:]])

            # ---- phase 0: Hhat1 = dis1*(X @ W1) per shard, AllGather ----
            remaining[0] = [set(st) for st in sec_tiles]
            remaining[1] = [set(st) for st in sec_tiles]
            for q in range(n_quads):
                tts = quad_tiles(q)
                nq = len(tts)
                hp = hpool.tile([P, QUAD * F_HID], f32, tag="hp")
                for k, tt in enumerate(tts):
                    nc.tensor.matmul(out=hp[:, k * F_HID:(k + 1) * F_HID],
                                     lhsT=xt_sb[:, tt * P:(tt + 1) * P],
                                     rhs=w1_sb[:], start=True, stop=True)
                    nc.scalar.activation(
                        out=hall[0][:, tt * P:(tt + 1) * P],
                        in_=hp[:, k * F_HID:(k + 1) * F_HID],
                        func=copyf, scale=sw_sb[0][:, tt:tt + 1])
                hsl = hall[0][:, tts[0] * P:(tts[-1] + 1) * P]
                store_h(0, q, hsl.rearrange("p (k f) -> p k f", f=P))

            # ---- aggregation layers ----
            # f32 accumulator for the section-0 sweep partials
            acc = cpool.tile([P, shard], f32, tag="acc")
            for li in (0, 1):
                lay = layouts[li]
                bcol = b1c_sb if li == 0 else b2c_sb
                w_next = w2_sb if li == 0 else wl_sb
                n_next = F_HID if li == 0 else F_OUT

                # SWDGE preps: descriptor generation runs EARLY on the Q7
                # pairs (the h_sec read dep is deferred to the per-section
                # trigger), so the transfers burst at SDMA drain rate the
                # moment the AllGather section lands.
                gbufs = {}

                def gather(ci):
                    if ci not in gbufs:
                        (s, b0, nblk) = lay.chunks[ci]
                        gb = gpool.tile([P, SUB_B, P], gdt, tag="gb")
                        ni = nblk * P
                        nc.gpsimd.dma_gather(
                            gb[:, :nblk, :], h_sec[li][s][:],
                            idx_sb[li][:, b0 * 8:b0 * 8 + nblk * 8],
                            ni, ni, P, single_packet=False,
                            queue_num=qctr[0] % N_QUEUES)
                        qctr[0] += 1
                        gbufs[ci] = gb
                    return gbufs[ci]

                # one-hot panels, built on first use
                pbufs = {}

                def panel(pi):
                    if pi not in pbufs:
                        b0, nb = lay.panels[pi]
                        mp = ppool.tile([P, MB * P], gdt, tag="m")
                        nc.vector.tensor_tensor(
                            out=mp[:, :nb * P].rearrange(
                                "p (b f) -> p b f", f=P),
                            in0=iota_sb[:, :nb * P].rearrange(
                                "p (b f) -> p b f", f=P),
                            in1=dl_sb[li][:, b0:b0 + nb].broadcast_to(
                                (P, nb, P)),
                            op=is_eq)
                        pbufs[pi] = mp
                    return pbufs[pi]

                def blocks_of(tt, sweep_s):
                    for (s, b0, nb) in lay.tile_segs[tt]:
                        if s == sweep_s:
                            for b in range(b0, b0 + nb):
                                yield b

                def emit_blocks(tt, sweep_s, aggp, sl, close):
                    """Emit matmuls for tile tt's sweep_s blocks; if close,
                    the last one gets stop=True.  Returns True if none."""
                    blks = list(blocks_of(tt, sweep_s))
                    for i, b in enumerate(blks):
                        ci = int(lay.blk2chunk[b])
                        jj = b - lay.chunks[ci][1]
                        pi = int(lay.blk2panel[b])
                        mp = panel(pi)
                        gb = gather(ci)
                        pj = b - lay.panels[pi][0]
                        nc.tensor.matmul(
                            out=aggp[:, sl], lhsT=gb[:, jj, :],
                            rhs=mp[:, pj * P:(pj + 1) * P],
                            start=(i == 0),
                            stop=(close and i == len(blks) - 1))
                    return len(blks) == 0

                secA, secB = (0, 1) if li == 0 else (1, 0)
                quad_order = (list(reversed(range(n_quads))) if li == 0
                              else list(range(n_quads)))
                # sweep A: first-available section -> f32 accumulator
                for q in quad_order:
                    tts = quad_tiles(q)
                    nq = len(tts)
                    aggp = aggpool.tile([P, QUAD * P], f32, tag="aggp")
                    for k, tt in enumerate(tts):
                        sl = slice(k * P, (k + 1) * P)
                        if emit_blocks(tt, secA, aggp, sl, close=True):
                            nc.vector.memset(aggp[:, sl], 0.0)
                    cols = slice(tts[0] * P, (tts[-1] + 1) * P)
                    nc.scalar.activation(out=acc[:, cols],
                                         in_=aggp[:, :nq * P], func=copyf)
                # sweep B: remaining section + self-loop + carry-in, finalize
                for q in quad_order:
                    tts = quad_tiles(q)
                    nq = len(tts)
                    aggp = aggpool.tile([P, QUAD * P], f32, tag="aggp")
                    for k, tt in enumerate(tts):
                        sl = slice(k * P, (k + 1) * P)
                        first = emit_blocks(tt, secB, aggp, sl, close=False)
                        nc.tensor.matmul(
                            out=aggp[:, sl],
                            lhsT=hall[li][:, tt * P:(tt + 1) * P],
                            rhs=ident_sb[:], start=first, stop=True)
                    cols = slice(tts[0] * P, (tts[-1] + 1) * P)
                    # add the section-0 carry-in, then dis[dst] scale
                    nc.vector.tensor_tensor(
                        out=aggp[:, :nq * P], in0=aggp[:, :nq * P],
                        in1=acc[:, cols], op=mybir.AluOpType.add)
                    dqq = opool.tile([P, QUAD * P], f32, tag="dqq")
                    nc.sync.dma_start(dqq[:, :nq * P], dq_d[li][:, cols])
                    nc.vector.tensor_tensor(
                        out=aggp[:, :nq * P], in0=aggp[:, :nq * P],
                        in1=dqq[:, :nq * P], op=mult)
                    outT = opool.tile([P, QUAD * P], gdt, tag="outT")
                    nc.scalar.activation(out=outT[:, :nq * P],
                                         in_=aggp[:, :nq * P],
                                         func=relu, bias=bcol[:])
                    hp2 = hpool.tile([P, QUAD * F_HID], f32, tag="hp")
                    for k, tt in enumerate(tts):
                        nc.tensor.matmul(
                            out=hp2[:, k * n_next:(k + 1) * n_next],
                            lhsT=outT[:, k * P:(k + 1) * P],
                            rhs=w_next[:], start=True, stop=True)
                    if li == 0:
                        for k, tt in enumerate(tts):
                            nc.scalar.activation(
                                out=hall[1][:, tt * P:(tt + 1) * P],
                                in_=hp2[:, k * F_HID:(k + 1) * F_HID],
                                func=copyf, scale=sw_sb[1][:, tt:tt + 1])
                        hsl = hall[1][:, tts[0] * P:(tts[-1] + 1) * P]
                        store_h(1, q, hsl.rearrange("p (k f) -> p k f", f=P))
                    else:
                        osb = opool.tile([P, QUAD * F_OUT], f32, tag="osb")
                        for k in range(nq):
                            nc.vector.tensor_tensor(
                                out=osb[:, k * F_OUT:(k + 1) * F_OUT],
                                in0=hp2[:, k * F_OUT:(k + 1) * F_OUT],
                                in1=blbc_sb[:], op=mybir.AluOpType.add)
                        rows = slice(tts[0] * P, (tts[-1] + 1) * P)
                        dst = out_d[rows, :].rearrange("(k p) f -> p k f", p=P)
                        nc.sync.dma_start(
                            dst, osb[:, :nq * F_OUT].rearrange(
                                "p (k f) -> p k f", f=F_OUT))

    nc.compile()
    return nc


# ---------------------------------------------------------------------------
# Entry point
# ---------------------------------------------------------------------------

def _run(x, edge_index, mask1, mask2, W1, b1, W2, b2, Wl, bl,
         n, n_pad, lo_limit=None):
    layouts, in_maps = _prepare(x, edge_index, mask1, mask2,
                                W1, b1, W2, b2, Wl, bl, n, n_pad)
    nc = _build(layouts, n_pad)
    res = run_bass_kernel_spmd(nc, in_maps, core_ids=list(range(N_CORES)))
    out = np.concatenate([res.results[c]["out"] for c in range(N_CORES)],
                         axis=0)
    return out[:n].astype(np.float32)


def kernel(x, edge_index, mask1, mask2, W1, b1, W2, b2, Wl, bl):
    n_pad = 50176  # 8 cores * 49 tiles * 128
    return _run(x, edge_index, mask1, mask2, W1, b1, W2, b2, Wl, bl,
                N_NODES, n_pad)
